# revision 1
# baseline (speedup 1.0000x reference)
"""Trainium2 Bass kernel for an 8-expert top-2 MoE layer.

Strategy (expert-parallel, per the sharding hint "all-to-all tokens by
top-k assignment"): the host computes the (tiny) gating matmul + softmax
+ top-2 routing, gathers each expert's assigned tokens, and ships one
expert per NeuronCore. Each core runs the heavy 2-layer MLP for its
expert over its assigned tokens (f32r matmuls on the PE array), applies
the gate weights on-device, and the host scatter-adds the two expert
contributions per token back together.

The MLP math runs fully transposed (tokens on the free dim) so that
 - W1/W2 slices feed the PE as stationary operands with no transposes,
 - the b1 bias + relu and (y + b2) * gate evictions are single fused
   DVE ops with per-partition scalars,
 - the per-token gate row is broadcast across partitions with one K=1
   matmul (ones[1,128]^T @ g[1,N] -> G[128,N]).

DMA-dispatch overhead (~0.6us per dma_start, serialized on the DGE
queue) is first-order here, so transfers are batched: weights are
shipped as eight j-strip (o-strip) tensors host-packed to [128, 8, 128]
so each strip is one DMA and gates exactly one accumulation group, x
arrives as one DMA per token tile (host-packed [128, 8, C]), and the
biases/gates land in one merged DMA each. Token tiles are 512 wide
(one fp32 PSUM bank) with a final 256-wide tile to trim padding (f32r
keeps full rate at free dim >= 256). A few dummy matmuls run in the
shadow of the initial DMA ramp to engage the PE clock-gate (HAM)
before the real matmuls arrive.
"""

import numpy as np

NUM_EXPERTS = 8
TOP_K = 2
D = 1024

_prog_cache = {}


def _plan_tiles(max_load):
    """Token-tile sizes covering max_load: 512s with a final 256 when it fits."""
    n256 = -(-max_load // 256)
    tiles = [512] * (n256 // 2)
    if n256 % 2 == 1:
        tiles.append(256)
    if not tiles:
        tiles = [256]
    return sum(tiles), tiles


def _build_program(tile_plan):
    """Build the per-core Bass program: one expert's MLP over C tokens."""
    from contextlib import ExitStack

    import concourse.tile as tile
    from concourse import bacc, mybir

    f32 = mybir.dt.float32
    f32r = mybir.dt.float32r
    ADD = mybir.AluOpType.add
    MAX = mybir.AluOpType.max
    MULT = mybir.AluOpType.mult

    C, tok_tiles = tile_plan

    nc = bacc.Bacc("TRN2", target_bir_lowering=False, debug=False,
                   num_devices=NUM_EXPERTS)

    # host-packed layouts (see _make_in_maps):
    #   xT:  [128, 8, C]      xT[p, d, c] = x_gathered[c, d*128+p]
    #   w1:  [8, 128, 8, 128] w1[j, p, d, r] = W1[d*128+p, j*128+r]
    #   w2:  [8, 128, 8, 128] w2[o, p, j, r] = W2[j*128+p, o*128+r]
    #   b1:  [128, 8]         b1[p, j] = b1[j*128+p]   (b2 same)
    #   yT:  [128, 8, C]      yT[p, o, c] = y[c, o*128+p] * gate[c]
    xT_d = nc.dram_tensor("xT", [128, 8, C], f32r, kind="ExternalInput").ap()
    w1_d = nc.dram_tensor("w1", [8, 128, 8, 128], f32r, kind="ExternalInput").ap()
    w2_d = nc.dram_tensor("w2", [8, 128, 8, 128], f32r, kind="ExternalInput").ap()
    bb_d = nc.dram_tensor("bb", [128, 16], f32, kind="ExternalInput").ap()
    go_d = nc.dram_tensor("go", [1, C + 128], f32r, kind="ExternalInput").ap()
    yT_d = nc.dram_tensor("yT", [128, 8, C], f32, kind="ExternalOutput").ap()

    with tile.TileContext(nc) as tc, ExitStack() as ctx:
        wpool = ctx.enter_context(tc.tile_pool(name="w", bufs=1))
        cpool = ctx.enter_context(tc.tile_pool(name="const", bufs=1))
        xpool = ctx.enter_context(tc.tile_pool(name="x", bufs=2))
        hpool = ctx.enter_context(tc.tile_pool(name="h", bufs=2))
        ypool = ctx.enter_context(tc.tile_pool(name="y", bufs=2))
        gpool = ctx.enter_context(tc.tile_pool(name="g", bufs=2))
        php = ctx.enter_context(tc.tile_pool(name="ph", bufs=3, space="PSUM"))
        pyp = ctx.enter_context(tc.tile_pool(name="py", bufs=3, space="PSUM"))
        pgp = ctx.enter_context(tc.tile_pool(name="pg", bufs=2, space="PSUM"))

        # tiny constants on the ACT DGE queue (parallel with the weight
        # stream on the SP queue), merged into single transfers:
        # bb = [b1 | b2] per-partition, go = [gate row | ones row]
        bb_sb = cpool.tile([128, 16], f32, tag="bb")
        nc.sync.dma_start(bb_sb[:], bb_d[:])
        b1_sb = bb_sb[:, 0:8]
        b2_sb = bb_sb[:, 8:16]
        go_sb = cpool.tile([1, C + 128], f32r, tag="go")
        nc.sync.dma_start(go_sb[:], go_d[:])
        g_sb = go_sb[:, 0:C]
        ones_sb = go_sb[:, C:C + 128]

        # PE warm-up in the shadow of the initial DMA ramp: ~4us of dummy
        # K=1 matmuls (gated only on the tiny g/ones transfers) keep the
        # HAM activity monitor busy so the real matmuls run at 2.4 GHz
        warm = pgp.tile([128, min(C, 512)], f32, tag="gps")
        for _ in range(6):
            nc.tensor.matmul(warm[:], ones_sb[:], g_sb[:, 0:min(C, 512)],
                             start=True, stop=True)

        # DMA emission in consumption order: w1 strip 0, then the first
        # token tile of x^T per d-block (the j=0 group's d-MMs start as each
        # block lands), then the remaining w1 strips (one gates each j-group)
        TT0 = tok_tiles[0]
        w1_sb = [None] * 8
        w1_first = wpool.tile([128, 8 * 128], f32r, tag="w1_0")
        nc.sync.dma_start(w1_first[:], w1_d[0])
        w1_sb[0] = w1_first
        x0a = xpool.tile([128, 4 * TT0], f32r, tag="x0a")
        nc.sync.dma_start(x0a[:], xT_d[:, 0:4, 0:TT0])
        x0b = xpool.tile([128, 4 * TT0], f32r, tag="x0b")
        nc.sync.dma_start(x0b[:], xT_d[:, 4:8, 0:TT0])
        x_sb0 = (x0a, x0b)
        for j in range(1, 8):
            w1_strip = wpool.tile([128, 8 * 128], f32r, tag=f"w1_{j}")
            nc.sync.dma_start(w1_strip[:], w1_d[j])
            w1_sb[j] = w1_strip

        # w2 o-strips next: strip o gates tile 0's layer-2 o-group, which
        # starts right after layer 1 (~the w1 stream), so these must not
        # queue behind the second x tile; the second x tile (needed only
        # when tile 0 fully finishes) slots in before the last strip
        x_tiles = [None] * len(tok_tiles)
        x_tiles[0] = x_sb0
        w2_sb = [None] * 8
        for o in range(8):
            if o == 6 and len(tok_tiles) > 1:
                TT1 = tok_tiles[1]
                x1a = xpool.tile([128, 4 * TT1], f32r, tag="x0a")
                nc.sync.dma_start(x1a[:], xT_d[:, 0:4, TT0:TT0 + TT1])
                x_tiles[1] = (x1a, None)
            if o == 7 and len(tok_tiles) > 1:
                TT1 = tok_tiles[1]
                x1b = xpool.tile([128, 4 * TT1], f32r, tag="x0b")
                nc.sync.dma_start(x1b[:], xT_d[:, 4:8, TT0:TT0 + TT1])
                x_tiles[1] = (x_tiles[1][0], x1b)
            w2_strip = wpool.tile([128, 8 * 128], f32r, tag=f"w2_{o}")
            nc.sync.dma_start(w2_strip[:], w2_d[o])
            w2_sb[o] = w2_strip

        tile_pos = np.cumsum([0] + tok_tiles).tolist()
        pos = 0
        for t, TT in enumerate(tok_tiles):
            tsl = slice(pos, pos + TT)

            # prefetch x for tile t+1 (tiles 0 and 1 already issued)
            nt = t + 1
            if nt < len(tok_tiles) and x_tiles[nt] is None:
                x_pref = xpool.tile([128, 8 * tok_tiles[nt]], f32r, tag="x")
                nc.sync.dma_start(
                    x_pref[:],
                    xT_d[:, :, tile_pos[nt]:tile_pos[nt] + tok_tiles[nt]])
                x_tiles[nt] = x_pref

            x_sb = x_tiles[t]

            def xs(d):
                if isinstance(x_sb, tuple):
                    half = x_sb[d // 4]
                    return half[:, (d % 4) * TT:(d % 4 + 1) * TT]
                return x_sb[:, d * TT:(d + 1) * TT]

            # broadcast gate row across partitions: G[p, n] = g[n]
            g_ps = pgp.tile([128, TT], f32, tag="gps")
            nc.tensor.matmul(g_ps[:], ones_sb[:], g_sb[:, tsl],
                             start=True, stop=True)
            g_bc = gpool.tile([128, TT], f32, tag="gbc")
            nc.vector.tensor_copy(g_bc[:], g_ps[:])

            # layer 1: h^T[j,:] = relu(sum_d W1[d,j]^T x^T[d,:] + b1[j])
            h_sb = []
            for j in range(8):
                ph = php.tile([128, TT], f32, tag="ph")
                for d in range(8):
                    nc.tensor.matmul(ph[:],
                                     w1_sb[j][:, d * 128:(d + 1) * 128],
                                     xs(d),
                                     start=(d == 0), stop=(d == 7))
                ht = hpool.tile([128, TT], f32r, tag=f"h{j}")
                nc.vector.tensor_scalar(ht[:], ph[:], b1_sb[:, j:j + 1], 0.0,
                                        op0=ADD, op1=MAX)
                h_sb.append(ht)

            # layer 2 + gate: y^T[o,:] = (sum_j W2[j,o]^T h^T[j,:] + b2[o]) * g
            for o in range(8):
                py = pyp.tile([128, TT], f32, tag="py")
                for j in range(8):
                    nc.tensor.matmul(py[:],
                                     w2_sb[o][:, j * 128:(j + 1) * 128],
                                     h_sb[j][:],
                                     start=(j == 0), stop=(j == 7))
                yt = ypool.tile([128, TT], f32, tag=f"y{o}")
                nc.vector.scalar_tensor_tensor(yt[:], py[:], b2_sb[:, o:o + 1],
                                               g_bc[:], op0=ADD, op1=MULT)
                nc.sync.dma_start(yT_d[:, o, tsl], yt[:])

            pos += TT

    nc.compile()
    return nc


def _route(x, Wg, bg):
    """Host gating: fp32 softmax + top-2, matching jax.lax.top_k semantics."""
    logits = x @ Wg + bg
    m = logits.max(axis=1, keepdims=True)
    e = np.exp(logits - m)
    gates = e / e.sum(axis=1, keepdims=True)
    # stable argsort on negated values = ties broken by lower index (jax)
    order = np.argsort(-gates, axis=1, kind="stable")[:, :TOP_K]
    return gates, order


def _pack_w(W):
    """[1024, 1024] -> [8, 128, 8, 128]: strip s, part p, rowtile d, col r."""
    # out[s, p, d, r] = W[d*128+p, s*128+r]
    return np.ascontiguousarray(
        W.reshape(8, 128, 8, 128).transpose(2, 1, 0, 3))


def _make_in_maps(x, W1, b1, W2, b2, gates, order, tok_lists, C):
    in_maps = []
    for e in range(NUM_EXPERTS):
        toks = tok_lists[e]
        ne = len(toks)
        xT_e = np.zeros((128, 8, C), dtype=np.float32)
        # xT_e[p, d, :ne] = x[toks, d*128+p].T
        xT_e[:, :, :ne] = x[toks].T.reshape(8, 128, ne).transpose(1, 0, 2)
        g_e = np.zeros((1, C), dtype=np.float32)
        g_e[0, :ne] = gates[toks, e]
        in_maps.append({
            "xT": xT_e,
            "w1": _pack_w(W1[e]),
            "w2": _pack_w(W2[e]),
            "bb": np.ascontiguousarray(np.concatenate(
                [b1[e].reshape(8, 128).T, b2[e].reshape(8, 128).T], axis=1)),
            "go": np.concatenate(
                [g_e, np.ones((1, 128), dtype=np.float32)], axis=1),
        })
    return in_maps


def kernel(x, W1, b1, W2, b2, Wg, bg):
    from concourse import bass_utils

    x = np.ascontiguousarray(np.asarray(x, dtype=np.float32))
    W1 = np.asarray(W1, dtype=np.float32)
    b1 = np.asarray(b1, dtype=np.float32)
    W2 = np.asarray(W2, dtype=np.float32)
    b2 = np.asarray(b2, dtype=np.float32)
    Wg = np.asarray(Wg, dtype=np.float32)
    bg = np.asarray(bg, dtype=np.float32)
    n = x.shape[0]

    gates, order = _route(x, Wg, bg)
    tok_lists = [np.where((order == e).any(axis=1))[0] for e in range(NUM_EXPERTS)]
    max_load = max(len(t) for t in tok_lists)
    C, tok_tiles = _plan_tiles(max_load)

    key = (C, tuple(tok_tiles))
    if key not in _prog_cache:
        _prog_cache[key] = _build_program((C, tok_tiles))
    nc = _prog_cache[key]

    in_maps = _make_in_maps(x, W1, b1, W2, b2, gates, order, tok_lists, C)
    res = bass_utils.run_bass_kernel_spmd(nc, in_maps, list(range(NUM_EXPERTS)))
    # yT result: [128, 8, C] -> y_e[c, o*128+p] = yT[p, o, c]
    yT_all = np.stack([res.results[e]["yT"] for e in range(NUM_EXPERTS)])

    # scatter-add the two expert contributions per token (already gated)
    slot = np.zeros((NUM_EXPERTS, n), dtype=np.int64)
    for e in range(NUM_EXPERTS):
        slot[e, tok_lists[e]] = np.arange(len(tok_lists[e]))
    rows = np.arange(n)
    # gather columns: result [n, 128, 8] -> reshape to [n, 1024]
    out = np.zeros((n, D), dtype=np.float32)
    for k in range(TOP_K):
        ek = order[:, k]
        picked = yT_all[ek, :, :, slot[ek, rows]]   # [n, 128, 8]
        out += picked.transpose(0, 2, 1).reshape(n, D)
    return out



# revision 5
# speedup vs baseline: 1.3681x; 1.3681x over previous
"""Trainium2 Bass kernel for an 8-expert top-2 MoE layer.

Strategy: expert-parallel with gate-magnitude-tiered fp8 DoubleRow matmuls.

Routing (host): gating matmul + softmax + top-2.  Every (token, expert)
pair is assigned a precision tier based on its gate weight g — the
pair's contribution to the output is g*y, so small-g pairs tolerate
proportionally more matmul error:

  tier  96: full residual-compensated fp8  (x_hi/x_lo, W_hi/W_lo, h_hi/h_lo)
  tier  80: x- and W-compensated, h single fp8
  tier  64: W-compensated only
  tier  32: single-pass fp8

All tiers run on the PE as fp8e4 (e4m3) DoubleRow matmuls: each MM
contracts K=256 at 0.5 cycles/output-column, so a pair costs
16 cycles/K-1024-term per layer.  Residual compensation appends extra
K-tiles to the same PSUM accumulation group: W ~ q8(W*s) + q8(residual)
recovers near-bf16 weight precision at fp8 speed; same for x and h.
Weight/x scales (32x for W1, 64x for W2) keep the residuals inside
e4m3's normal range; the combined 2048x scale is divided out in the
final eviction (b2 pre-scaled, gates applied on host).

Scheduling: expert e's pairs live on cores e (slot A) and e-1 (slot B).
Each expert's pairs are sorted by g and refilled into uniform per-tier
quotas C_t = max_e n_t(e); borderline pairs get promoted into slack
slots (free accuracy), residual slack becomes zero-gate padding in the
cheapest tier.  The SPMD program is identical on all cores: per-tier
segments of fixed length, slot-A part then slot-B part, chopped into
<=512-token tiles (one PSUM bank).

Per tile: L1 strips j=0..7 accumulate 4-12 DR MMs into PSUM, ACT engine
evicts relu(psum + 32*b1) straight to fp8 (plus f32 + residual evictions
for tier 96); L2 strips o=0..7 accumulate and DVE evicts
(psum + 2048*b2) * (1/2048) to bf16.  Host applies gates and
scatter-adds the two expert contributions per token.
"""

import numpy as np
import ml_dtypes

NUM_EXPERTS = 8
TOP_K = 2
D = 1024

# gate-threshold ladder: pair gets the highest tier whose threshold its
# gate exceeds.  sigma-matched: th_t ~ K_BUDGET / sigma_t with measured
# per-pair error sigma {32: 7.15e-3, 64: 5.33e-3, 80: 3.79e-3}.
TH32 = 0.140
TH64 = 0.188
TH80 = 0.264

TIERS = (96, 80, 64, 32)     # segment order on the token axis
TILE = 512                   # PSUM bank width in fp32

_prog_cache = {}

f8np = ml_dtypes.float8_e4m3
bf16np = ml_dtypes.bfloat16


def _q8(a):
    return a.astype(f8np).astype(np.float32)


def _route(x, Wg, bg):
    """Host gating: fp32 softmax + top-2, matching jax.lax.top_k semantics."""
    logits = x @ Wg + bg
    m = logits.max(axis=1, keepdims=True)
    e = np.exp(logits - m)
    gates = e / e.sum(axis=1, keepdims=True)
    order = np.argsort(-gates, axis=1, kind="stable")[:, :TOP_K]
    return gates, order


def _schedule(gates, order):
    """Tier pairs by gate, refill per-expert quotas, split A/B parts.

    Returns (plan_key, per_expert) where per_expert[e] is a dict
    tier -> (tokens, gvals) arrays of length C_t (token -1 = padding).
    """
    n = gates.shape[0]
    per_expert_pairs = []
    for e in range(NUM_EXPERTS):
        toks = np.where((order == e).any(axis=1))[0]
        g = gates[toks, e]
        srt = np.argsort(-g, kind="stable")
        per_expert_pairs.append((toks[srt], g[srt]))

    def req_tier(g):
        t = np.full(g.shape, 96, np.int64)
        t[g < TH80] = 80
        t[g < TH64] = 64
        t[g < TH32] = 32
        return t

    counts = {t: [] for t in TIERS}
    for e in range(NUM_EXPERTS):
        t = req_tier(per_expert_pairs[e][1])
        for tier in TIERS:
            counts[tier].append(int((t == tier).sum()))
    C = {t: max(counts[t]) for t in TIERS}

    per_expert = []
    for e in range(NUM_EXPERTS):
        toks, g = per_expert_pairs[e]
        segs = {}
        pos = 0
        for tier in TIERS:
            ct = C[tier]
            tt = np.full(ct, -1, np.int64)
            gg = np.zeros(ct, np.float64)
            take = max(0, min(ct, len(toks) - pos))
            if take > 0:
                tt[:take] = toks[pos:pos + take]
                gg[:take] = g[pos:pos + take]
            segs[tier] = (tt, gg)
            pos += take
        assert pos == len(toks), (pos, len(toks))
        per_expert.append(segs)

    A = {t: C[t] // 2 for t in TIERS}
    key = tuple(C[t] for t in TIERS) + tuple(A[t] for t in TIERS)
    return key, C, A, per_expert


def _plan_tiles(C, A):
    """Chop the token axis into single-(tier,slot) tiles of <=TILE tokens.

    Returns list of (tier, slot, seg_lo, width) where seg_lo is the tile's
    offset from the start of the full token axis.
    """
    tiles = []
    base = 0
    for tier in TIERS:
        for slot, lo, hi in ((0, 0, A[tier]), (1, A[tier], C[tier])):
            p = lo
            while p < hi:
                w = min(TILE, hi - p)
                tiles.append((tier, slot, base + p, w))
                p += w
        base += C[tier]
    return tiles


def _build_program(key):
    from contextlib import ExitStack

    import concourse.tile as tile
    from concourse import bacc, mybir

    f32 = mybir.dt.float32
    f32r = mybir.dt.float32r
    f8 = mybir.dt.float8e4
    bf16 = mybir.dt.bfloat16
    ADD = mybir.AluOpType.add
    MULT = mybir.AluOpType.mult
    SUB = mybir.AluOpType.subtract
    RELU = mybir.ActivationFunctionType.Relu
    DR = mybir.MatmulPerfMode.DoubleRow

    C = dict(zip(TIERS, key[:4]))
    A = dict(zip(TIERS, key[4:]))
    CA = sum(C.values())
    CH = C[96] + C[80]            # prefix that carries x residuals
    tiles = _plan_tiles(C, A)

    nc = bacc.Bacc("TRN2", target_bir_lowering=False, debug=False,
                   num_devices=NUM_EXPERTS)

    x8_d = nc.dram_tensor("x8", [128, 4, 2, CA], f8, kind="ExternalInput").ap()
    rx8_d = (nc.dram_tensor("rx8", [128, 4, 2, max(CH, 16)], f8,
                            kind="ExternalInput").ap())
    w_d = {}
    for nm in ("w1h", "w1l", "w2h", "w2l"):
        w_d[nm] = nc.dram_tensor(nm, [2, 128, 4, 8, 2, 128], f8,
                                 kind="ExternalInput").ap()
    # bb[p, slot, 0:8]=32*b1 ; bb[p, slot, 8:16]=2048*b2
    bb_d = nc.dram_tensor("bb", [128, 2, 16], f32, kind="ExternalInput").ap()
    wrm_d = nc.dram_tensor("wrm", [1, 640], f32r, kind="ExternalInput").ap()
    y_d = nc.dram_tensor("y", [128, 8, CA], bf16, kind="ExternalOutput").ap()

    with tile.TileContext(nc) as tc, ExitStack() as ctx:
        wpool = ctx.enter_context(tc.tile_pool(name="w", bufs=1))
        cpool = ctx.enter_context(tc.tile_pool(name="const", bufs=1))
        xpool = ctx.enter_context(tc.tile_pool(name="x", bufs=2))
        rxpool = ctx.enter_context(tc.tile_pool(name="rx", bufs=2))
        hpool = ctx.enter_context(tc.tile_pool(name="h", bufs=2))
        rhpool = ctx.enter_context(tc.tile_pool(name="rh", bufs=2))
        fpool = ctx.enter_context(tc.tile_pool(name="hf", bufs=3))
        ypool = ctx.enter_context(tc.tile_pool(name="y", bufs=2))
        php = ctx.enter_context(tc.tile_pool(name="ph", bufs=3, space="PSUM"))
        pyp = ctx.enter_context(tc.tile_pool(name="py", bufs=3, space="PSUM"))
        pwp = ctx.enter_context(tc.tile_pool(name="pw", bufs=1, space="PSUM"))

        # constants first (small, unblock warmup + evictions)
        wrm_sb = cpool.tile([1, 640], f32r, tag="wrm")
        nc.sync.dma_start(wrm_sb[:], wrm_d[:])
        bb_sb = cpool.tile([128, 2, 16], f32, tag="bb")
        nc.sync.dma_start(bb_sb[:], bb_d[:])

        # PE warm-up in the DMA shadow: ~4us of K=1 f32r matmuls keeps the
        # clock-ramp model at full speed by the time real matmuls arrive
        warm = pwp.tile([128, 512], f32, tag="warm")
        for _ in range(10):
            nc.tensor.matmul(warm[:], wrm_sb[:, 0:128], wrm_sb[:, 128:640],
                             start=True, stop=True)

        # weight tiles: [slot][name] -> [128, 4, 8, 2, 128]
        w_sb = [{}, {}]
        w1_first = wpool.tile([128, 4, 8, 2, 128], f8, tag="w1h0")
        nc.sync.dma_start(w1_first[:], w_d["w1h"][0])
        w_sb[0]["w1h"] = w1_first
        w1l_first = wpool.tile([128, 4, 8, 2, 128], f8, tag="w1l0")
        nc.sync.dma_start(w1l_first[:], w_d["w1l"][0])
        w_sb[0]["w1l"] = w1l_first

        # x/rx tiles DMA'd per tile, two tiles ahead of consumption.
        x_tiles = [None] * len(tiles)
        rx_tiles = [None] * len(tiles)

        def fetch(ti):
            tier, slot, lo, w = tiles[ti]
            wp = -(-w // 16) * 16
            xt = xpool.tile([128, 4, 2, wp], f8, tag="x")
            nc.sync.dma_start(xt[:, :, :, 0:w], x8_d[:, :, :, lo:lo + w])
            x_tiles[ti] = xt
            if tier >= 80:
                rt = rxpool.tile([128, 4, 2, wp], f8, tag="rx")
                nc.sync.dma_start(rt[:, :, :, 0:w], rx8_d[:, :, :, lo:lo + w])
                rx_tiles[ti] = rt

        fetch(0)
        # rest of slot-A weights, then prefetch tile 1, then slot-B weights
        for nm in ("w2h", "w2l"):
            t = wpool.tile([128, 4, 8, 2, 128], f8, tag=f"{nm}0")
            nc.sync.dma_start(t[:], w_d[nm][0])
            w_sb[0][nm] = t
        if len(tiles) > 1:
            fetch(1)
        for nm in ("w1h", "w1l", "w2h", "w2l"):
            t = wpool.tile([128, 4, 8, 2, 128], f8, tag=f"{nm}1")
            nc.sync.dma_start(t[:], w_d[nm][1])
            w_sb[1][nm] = t

        for ti, (tier, slot, lo, w) in enumerate(tiles):
            if ti + 2 < len(tiles):
                fetch(ti + 2)
            wp = -(-w // 16) * 16
            xt = x_tiles[ti]
            rt = rx_tiles[ti]
            ws = w_sb[slot]
            b1c = bb_sb[:, slot, 0:8]
            b2c = bb_sb[:, slot, 8:16]

            # layer 1
            h8 = hpool.tile([128, 4, 2, wp], f8, tag="h8")
            rh8 = None
            if tier >= 96:
                rh8 = rhpool.tile([128, 4, 2, wp], f8, tag="rh8", name="rh8")
            for j in range(8):
                ph = php.tile([128, w], f32, tag="ph")
                terms = [("w1h", xt)]
                if tier >= 64:
                    terms.append(("w1l", xt))
                if tier >= 80:
                    terms.append(("w1h", rt))
                nmm = len(terms) * 4
                i = 0
                for wname, mv in terms:
                    for kt in range(4):
                        nc.tensor.matmul(ph[:], ws[wname][:, kt, j],
                                         mv[:, kt, :, 0:w],
                                         start=(i == 0), stop=(i == nmm - 1),
                                         perf_mode=DR)
                        i += 1
                dst = h8[:, j // 2, j % 2, 0:w]
                if tier < 96:
                    nc.scalar.activation(dst, ph[:], RELU, bias=b1c[:, j:j + 1])
                else:
                    hf = fpool.tile([128, w], f32, tag="hf")
                    nc.scalar.activation(hf[:], ph[:], RELU, bias=b1c[:, j:j + 1])
                    nc.vector.tensor_copy(dst, hf[:])
                    nc.vector.tensor_sub(rh8[:, j // 2, j % 2, 0:w],
                                         hf[:], dst)

            # layer 2
            yt = ypool.tile([128, 8, wp], bf16, tag="yt")
            for o in range(8):
                py = pyp.tile([128, w], f32, tag="py")
                terms = [("w2h", h8)]
                if tier >= 64:
                    terms.append(("w2l", h8))
                if tier >= 96:
                    terms.append(("w2h", rh8))
                nmm = len(terms) * 4
                i = 0
                for wname, mv in terms:
                    for kt in range(4):
                        nc.tensor.matmul(py[:], ws[wname][:, kt, o],
                                         mv[:, kt, :, 0:w],
                                         start=(i == 0), stop=(i == nmm - 1),
                                         perf_mode=DR)
                        i += 1
                nc.vector.tensor_scalar(yt[:, o, 0:w], py[:], b2c[:, o:o + 1],
                                        1.0 / 2048.0, op0=ADD, op1=MULT)
            nc.sync.dma_start(y_d[:, :, lo:lo + w], yt[:, :, 0:w])

    nc.compile()
    return nc


def _pack_w_dr(Wt, scale):
    """[1024,1024] -> hi,lo packed [128, 4, 8, 2, 128] fp8 for DR matmuls.

    packed[p, kt, j, i, m] = W[kt*256 + i*128 + p, j*128 + m] * scale
    """
    Ws = (Wt * scale).astype(np.float32)
    Wh = _q8(Ws)
    Wl = Ws - Wh
    def pack(a):
        # [1024, 1024] -> [4, 2, 128, 8, 128] -> [128, 4, 8, 2, 128]
        b = a.reshape(4, 2, 128, 8, 128).transpose(2, 0, 3, 1, 4)
        return np.ascontiguousarray(b).astype(f8np)
    return pack(Wh), pack(Wl)


def _pack_x_dr(xg):
    """[n, 1024] tokens -> [128, 4, 2, n] fp8 (+ residual packed same way)."""
    n = xg.shape[0]
    x8 = xg.astype(f8np).astype(np.float32)
    rx = xg - x8
    def pack(a):
        # [n, 1024] -> [n, 4, 2, 128] -> [128, 4, 2, n]
        b = a.reshape(n, 4, 2, 128).transpose(3, 1, 2, 0)
        return np.ascontiguousarray(b).astype(f8np)
    return pack(x8), pack(rx)


def kernel(x, W1, b1, W2, b2, Wg, bg):
    from concourse import bass_utils

    x = np.ascontiguousarray(np.asarray(x, dtype=np.float32))
    W1 = np.asarray(W1, dtype=np.float32)
    b1 = np.asarray(b1, dtype=np.float32)
    W2 = np.asarray(W2, dtype=np.float32)
    b2 = np.asarray(b2, dtype=np.float32)
    Wg = np.asarray(Wg, dtype=np.float32)
    bg = np.asarray(bg, dtype=np.float32)
    n = x.shape[0]

    gates, order = _route(x, Wg, bg)
    key, C, A, per_expert = _schedule(gates, order)
    if key not in _prog_cache:
        _prog_cache[key] = _build_program(key)
    nc = _prog_cache[key]

    CA = sum(C.values())
    CH = C[96] + C[80]

    # per-core token layout: for each tier segment, slot-A tokens from
    # expert c then slot-B tokens from expert (c+1) % 8
    in_maps = []
    core_layout = []   # per core: list of (expert, token_ids, gvals) in order
    for c in range(NUM_EXPERTS):
        toks_order = []
        layout = []
        for tier in TIERS:
            ta, ga = per_expert[c][tier]
            tb, gb = per_expert[(c + 1) % NUM_EXPERTS][tier]
            a = A[tier]
            layout.append((c, ta[:a], ga[:a]))
            layout.append(((c + 1) % NUM_EXPERTS, tb[a:], gb[a:]))
            toks_order.append(ta[:a])
            toks_order.append(tb[a:])
        toks = np.concatenate(toks_order)
        assert len(toks) == CA
        xg = np.zeros((CA, D), np.float32)
        real = toks >= 0
        xg[real] = x[toks[real]]
        x8p, rx8p = _pack_x_dr(xg)
        w1h_a, w1l_a = _pack_w_dr(W1[c], 32.0)
        w2h_a, w2l_a = _pack_w_dr(W2[c], 64.0)
        cb = (c + 1) % NUM_EXPERTS
        w1h_b, w1l_b = _pack_w_dr(W1[cb], 32.0)
        w2h_b, w2l_b = _pack_w_dr(W2[cb], 64.0)
        bb = np.zeros((128, 2, 16), np.float32)
        for s, e in ((0, c), (1, cb)):
            bb[:, s, 0:8] = 32.0 * b1[e].reshape(8, 128).T
            bb[:, s, 8:16] = 2048.0 * b2[e].reshape(8, 128).T
        in_maps.append({
            "x8": x8p,
            "rx8": np.ascontiguousarray(rx8p[:, :, :, :max(CH, 16)]),
            "w1h": np.stack([w1h_a, w1h_b]),
            "w1l": np.stack([w1l_a, w1l_b]),
            "w2h": np.stack([w2h_a, w2h_b]),
            "w2l": np.stack([w2l_a, w2l_b]),
            "bb": bb,
            "wrm": np.ones((1, 640), np.float32),
        })
        core_layout.append(layout)

    res = bass_utils.run_bass_kernel_spmd(nc, in_maps, list(range(NUM_EXPERTS)))

    out = np.zeros((n, D), np.float32)
    for c in range(NUM_EXPERTS):
        yv = np.asarray(res.results[c]["y"]).astype(np.float32)  # [128, 8, CA]
        pos = 0
        for e, toks, gv in core_layout[c]:
            m = toks >= 0
            if m.any():
                idx = np.nonzero(m)[0] + pos
                # yv[p, o, idx] -> [len, 1024] with d = o*128 + p
                picked = yv[:, :, idx]                  # [128, 8, len]
                contrib = picked.transpose(2, 1, 0).reshape(-1, D)
                out[toks[m]] += gv[m][:, None].astype(np.float32) * contrib
            pos += len(toks)
        assert pos == CA
    return out


# revision 6
# speedup vs baseline: 1.5374x; 1.1238x over previous
"""Trainium2 Bass kernel for an 8-expert top-2 MoE layer.

Strategy: expert-parallel with gate-magnitude-tiered fp8 DoubleRow matmuls.

Routing (host): gating matmul + softmax + top-2.  Every (token, expert)
pair is assigned a precision tier based on its gate weight g — the
pair's contribution to the output is g*y, so small-g pairs tolerate
proportionally more matmul error:

  tier  96: full residual-compensated fp8  (x_hi/x_lo, W_hi/W_lo, h_hi/h_lo)
  tier  80: x- and W-compensated, h single fp8
  tier  64: W-compensated only
  tier  32: single-pass fp8

All tiers run on the PE as fp8e4 (e4m3) DoubleRow matmuls: each MM
contracts K=256 at 0.5 cycles/output-column.  Residual compensation
appends extra K-tiles to the same PSUM accumulation group:
W ~ q8(W*s) + q8(residual) recovers near-bf16 weight precision at fp8
speed; same for x and h.  Weight scales (32x W1, 64x W2) keep residuals
in e4m3's normal range; the 2048x product is divided out at eviction
(b2 pre-scaled, gates applied on host).

Scheduling: expert e's pairs live on cores e (slot A) and e-1 (slot B).
Each expert's pairs are sorted by g and refilled into uniform per-tier
quotas sized by prefix-max over experts (so per-expert slack lands in
the cheapest tier and borderline pairs get free accuracy promotions).
The SPMD program is identical on all cores: per-tier segments of fixed
length, slot-A part then slot-B part, chopped into <=512-token tiles.

Per tile: L1 strips j=0..7 accumulate 4-12 DR MMs into one PSUM bank,
ACT engine evicts relu(psum + 32*b1) straight to fp8 (plus f32 +
residual for tier 96); L2 strips o=0..7 accumulate and DVE evicts
(psum + 2048*b2) * (1/2048) to bf16.  Host applies gates and
scatter-adds the two expert contributions per token.

DMA-cost notes: x/rx/y use per-tile contiguous blocks (descriptors
>=512B avoid the sub-512B half-rate penalty), slot-A W1 hi/lo stream in
j-strip chunks so layer-1 matmuls start ~2.7us in, and slot-B weights
arrive in the shadow of slot-A compute.
"""

import numpy as np
import ml_dtypes

NUM_EXPERTS = 8
TOP_K = 2
D = 1024

# gate-threshold ladder: pair gets the highest tier whose threshold its
# gate exceeds.  sigma-matched: th_t ~ K_BUDGET / sigma_t with measured
# per-pair error sigma {32: 7.15e-3, 64: 5.33e-3, 80: 3.79e-3}.
TH32 = 0.140
TH64 = 0.188
TH80 = 0.264

TIERS = (96, 80, 64, 32)     # segment order on the token axis
L1_TERMS = {96: 3, 80: 3, 64: 2, 32: 1}
L2_TERMS = {96: 3, 80: 2, 64: 2, 32: 1}
TILE = 512                   # PSUM bank width in fp32
NWARM = 5

_prog_cache = {}

f8np = ml_dtypes.float8_e4m3
bf16np = ml_dtypes.bfloat16


def _q8(a):
    return a.astype(f8np).astype(np.float32)


def _pad16(v):
    return -(-v // 16) * 16


def _route(x, Wg, bg):
    """Host gating: fp32 softmax + top-2, matching jax.lax.top_k semantics."""
    logits = x @ Wg + bg
    m = logits.max(axis=1, keepdims=True)
    e = np.exp(logits - m)
    gates = e / e.sum(axis=1, keepdims=True)
    order = np.argsort(-gates, axis=1, kind="stable")[:, :TOP_K]
    return gates, order


def _schedule(gates, order):
    """Tier pairs by gate, size quotas by prefix-max, refill per expert."""
    per_expert_pairs = []
    for e in range(NUM_EXPERTS):
        toks = np.where((order == e).any(axis=1))[0]
        g = gates[toks, e]
        srt = np.argsort(-g, kind="stable")
        per_expert_pairs.append((toks[srt], g[srt]))

    # prefix-max quota sizing: C_t chosen so every prefix sum dominates
    # every expert's required prefix; per-expert slack lands at tier 32
    prefix_req = {t: 0 for t in TIERS}
    for e in range(NUM_EXPERTS):
        g = per_expert_pairs[e][1]
        req = np.full(g.shape, 96, np.int64)
        req[g < TH80] = 80
        req[g < TH64] = 64
        req[g < TH32] = 32
        acc = 0
        for tier in TIERS:
            acc += int((req == tier).sum())
            prefix_req[tier] = max(prefix_req[tier], acc)

    C = {}
    acc = 0
    for tier in TIERS:
        C[tier] = _pad16(max(prefix_req[tier] - acc, 0))
        acc += C[tier]

    per_expert = []
    for e in range(NUM_EXPERTS):
        toks, g = per_expert_pairs[e]
        segs = {}
        pos = 0
        for tier in TIERS:
            ct = C[tier]
            tt = np.full(ct, -1, np.int64)
            gg = np.zeros(ct, np.float64)
            take = max(0, min(ct, len(toks) - pos))
            if take > 0:
                tt[:take] = toks[pos:pos + take]
                gg[:take] = g[pos:pos + take]
            segs[tier] = (tt, gg)
            pos += take
        assert pos == len(toks), (pos, len(toks))
        per_expert.append(segs)

    A = {t: (C[t] // 32) * 16 for t in TIERS}
    key = tuple(C[t] for t in TIERS) + tuple(A[t] for t in TIERS)
    return key, C, A, per_expert


def _plan_tiles(C, A):
    """Emit-ordered tiles: (tier, slot, tok_lo, width, xoff, rxoff, yoff).

    tok_lo indexes the CA token axis (tier segments in TIERS order, slot
    A part then B part).  xoff/yoff are element offsets (per partition)
    into the per-tile-block x8/y dram tensors; rxoff likewise for tiers
    >= 80 (else -1).
    """
    seg_base = {}
    base = 0
    for tier in TIERS:
        seg_base[tier] = base
        base += C[tier]

    def seg_tiles(tier, slot):
        lo, hi = (0, A[tier]) if slot == 0 else (A[tier], C[tier])
        out = []
        p = lo
        while p < hi:
            w = min(TILE, hi - p)
            out.append((tier, slot, seg_base[tier] + p, w))
            p += w
        return out

    emit = []
    for slot in (0, 1):
        for tier in TIERS:
            emit += seg_tiles(tier, slot)

    tiles = []
    xoff = rxoff = yoff = 0
    for tier, slot, lo, w in emit:
        r = rxoff if tier >= 80 else -1
        tiles.append((tier, slot, lo, w, xoff, r, yoff))
        xoff += 8 * w
        yoff += 8 * w
        if tier >= 80:
            rxoff += 8 * w
    return tiles, xoff, max(rxoff, 16)


def _build_program(key):
    from contextlib import ExitStack

    import concourse.tile as tile
    from concourse import bacc, mybir

    f32 = mybir.dt.float32
    f32r = mybir.dt.float32r
    f8 = mybir.dt.float8e4
    bf16 = mybir.dt.bfloat16
    ADD = mybir.AluOpType.add
    MULT = mybir.AluOpType.mult
    RELU = mybir.ActivationFunctionType.Relu
    DR = mybir.MatmulPerfMode.DoubleRow

    C = dict(zip(TIERS, key[:4]))
    A = dict(zip(TIERS, key[4:]))
    tiles, XT, RXT = _plan_tiles(C, A)

    nc = bacc.Bacc("TRN2", target_bir_lowering=False, debug=False,
                   num_devices=NUM_EXPERTS)

    x8_d = nc.dram_tensor("x8", [128, XT], f8, kind="ExternalInput").ap()
    rx8_d = nc.dram_tensor("rx8", [128, RXT], f8, kind="ExternalInput").ap()
    w_d = {}
    for nm in ("w1h", "w1l", "w2h", "w2l"):
        # [slot][p][j][kt*2*128]
        w_d[nm] = nc.dram_tensor(nm, [2, 128, 8, 1024], f8,
                                 kind="ExternalInput").ap()
    # bb[p, slot, 0:8]=32*b1 ; bb[p, slot, 8:16]=2048*b2
    bb_d = nc.dram_tensor("bb", [128, 2, 16], f32, kind="ExternalInput").ap()
    wrm_d = nc.dram_tensor("wrm", [1, 640], f32r, kind="ExternalInput").ap()
    y_d = nc.dram_tensor("y", [128, XT], bf16, kind="ExternalOutput").ap()

    with tile.TileContext(nc) as tc, ExitStack() as ctx:
        wpool = ctx.enter_context(tc.tile_pool(name="w", bufs=1))
        cpool = ctx.enter_context(tc.tile_pool(name="const", bufs=1))
        xpool = ctx.enter_context(tc.tile_pool(name="x", bufs=2))
        rxpool = ctx.enter_context(tc.tile_pool(name="rx", bufs=2))
        hpool = ctx.enter_context(tc.tile_pool(name="h", bufs=2))
        rhpool = ctx.enter_context(tc.tile_pool(name="rh", bufs=2))
        fpool = ctx.enter_context(tc.tile_pool(name="hf", bufs=3))
        ypool = ctx.enter_context(tc.tile_pool(name="y", bufs=2))
        php = ctx.enter_context(tc.tile_pool(name="ph", bufs=3, space="PSUM"))
        pyp = ctx.enter_context(tc.tile_pool(name="py", bufs=3, space="PSUM"))
        pwp = ctx.enter_context(tc.tile_pool(name="pw", bufs=1, space="PSUM"))

        # constants first (small, unblock warmup + evictions)
        wrm_sb = cpool.tile([1, 640], f32r, tag="wrm")
        nc.sync.dma_start(wrm_sb[:], wrm_d[:])
        bb_sb = cpool.tile([128, 2, 16], f32, tag="bb")
        nc.sync.dma_start(bb_sb[:], bb_d[:])

        # PE warm-up in the DMA shadow: K=1 f32r matmuls engage the
        # clock-ramp model before the real matmuls arrive
        warm = pwp.tile([128, 512], f32, tag="warm")
        for _ in range(NWARM):
            nc.tensor.matmul(warm[:], wrm_sb[:, 0:128], wrm_sb[:, 128:640],
                             start=True, stop=True)

        # weight tiles: [slot][name] -> [128, 8, 4, 2, 128]
        w_sb = [{}, {}]
        for s in (0, 1):
            for nm in ("w1h", "w1l", "w2h", "w2l"):
                w_sb[s][nm] = wpool.tile([128, 8, 4, 2, 128], f8,
                                         tag=f"{nm}{s}", name=f"{nm}{s}")

        x_tiles = [None] * len(tiles)
        rx_tiles = [None] * len(tiles)

        def fetch(ti):
            tier, slot, lo, w, xo, rxo, yo = tiles[ti]
            xt = xpool.tile([128, 4, 2, w], f8, tag="x", name="xt")
            nc.sync.dma_start(xt[:], x8_d[:, xo:xo + 8 * w])
            x_tiles[ti] = xt
            if tier >= 80:
                rt = rxpool.tile([128, 4, 2, w], f8, tag="rx", name="rxt")
                nc.sync.dma_start(rt[:], rx8_d[:, rxo:rxo + 8 * w])
                rx_tiles[ti] = rt

        # slot-A W1 hi/lo in interleaved j-pair chunks, first x tile early,
        # then the rest of slot A, then slot B whole
        for jj in range(4):
            nc.sync.dma_start(w_sb[0]["w1h"][:, 2 * jj:2 * jj + 2],
                              w_d["w1h"][0][:, 2 * jj:2 * jj + 2])
            nc.sync.dma_start(w_sb[0]["w1l"][:, 2 * jj:2 * jj + 2],
                              w_d["w1l"][0][:, 2 * jj:2 * jj + 2])
            if jj == 0:
                fetch(0)
        for nm in ("w2h", "w2l"):
            nc.sync.dma_start(w_sb[0][nm][:], w_d[nm][0])
        if len(tiles) > 1:
            fetch(1)
        for nm in ("w1h", "w1l", "w2h", "w2l"):
            nc.sync.dma_start(w_sb[1][nm][:], w_d[nm][1])

        for ti, (tier, slot, lo, w, xo, rxo, yo) in enumerate(tiles):
            if ti + 2 < len(tiles):
                fetch(ti + 2)
            xt = x_tiles[ti]
            rt = rx_tiles[ti]
            ws = w_sb[slot]
            b1c = bb_sb[:, slot, 0:8]
            b2c = bb_sb[:, slot, 8:16]

            # layer 1
            h8 = hpool.tile([128, 4, 2, w], f8, tag="h8", name="h8")
            rh8 = None
            if tier >= 96:
                rh8 = rhpool.tile([128, 4, 2, w], f8, tag="rh8", name="rh8")
            for j in range(8):
                ph = php.tile([128, w], f32, tag="ph", name="ph")
                terms = [("w1h", xt)]
                if tier >= 64:
                    terms.append(("w1l", xt))
                if tier >= 80:
                    terms.append(("w1h", rt))
                nmm = len(terms) * 4
                i = 0
                for wname, mv in terms:
                    for kt in range(4):
                        nc.tensor.matmul(ph[:], ws[wname][:, j, kt],
                                         mv[:, kt],
                                         start=(i == 0), stop=(i == nmm - 1),
                                         perf_mode=DR)
                        i += 1
                dst = h8[:, j // 2, j % 2]
                if tier < 96:
                    nc.scalar.activation(dst, ph[:], RELU, bias=b1c[:, j:j + 1])
                else:
                    hf = fpool.tile([128, w], f32, tag="hf", name="hf")
                    nc.scalar.activation(hf[:], ph[:], RELU, bias=b1c[:, j:j + 1])
                    nc.vector.tensor_copy(dst, hf[:])
                    nc.vector.tensor_sub(rh8[:, j // 2, j % 2], hf[:], dst)

            # layer 2
            yt = ypool.tile([128, 8, w], bf16, tag="yt", name="yt")
            for o in range(8):
                py = pyp.tile([128, w], f32, tag="py", name="py")
                terms = [("w2h", h8)]
                if tier >= 64:
                    terms.append(("w2l", h8))
                if tier >= 96:
                    terms.append(("w2h", rh8))
                nmm = len(terms) * 4
                i = 0
                for wname, mv in terms:
                    for kt in range(4):
                        nc.tensor.matmul(py[:], ws[wname][:, o, kt],
                                         mv[:, kt],
                                         start=(i == 0), stop=(i == nmm - 1),
                                         perf_mode=DR)
                        i += 1
                nc.vector.tensor_scalar(yt[:, o], py[:], b2c[:, o:o + 1],
                                        1.0 / 2048.0, op0=ADD, op1=MULT)
            nc.sync.dma_start(y_d[:, yo:yo + 8 * w], yt[:])

    nc.compile()
    return nc


def _pack_w_dr(Wt, scale):
    """[1024,1024] -> hi,lo packed [128, 8, 1024] fp8 for DR matmuls.

    packed[p, j, kt*256 + i*128 + m] = W[kt*256 + i*128 + p, j*128 + m] * scale
    """
    Ws = (Wt * scale).astype(np.float32)
    Wh = _q8(Ws)
    Wl = Ws - Wh
    def pack(a):
        # [1024, 1024] -> [4, 2, 128, 8, 128] -> [128, 8, 4, 2, 128]
        b = a.reshape(4, 2, 128, 8, 128).transpose(2, 3, 0, 1, 4)
        return np.ascontiguousarray(b).reshape(128, 8, 1024).astype(f8np)
    return pack(Wh), pack(Wl)


def kernel(x, W1, b1, W2, b2, Wg, bg):
    from concourse import bass_utils

    x = np.ascontiguousarray(np.asarray(x, dtype=np.float32))
    W1 = np.asarray(W1, dtype=np.float32)
    b1 = np.asarray(b1, dtype=np.float32)
    W2 = np.asarray(W2, dtype=np.float32)
    b2 = np.asarray(b2, dtype=np.float32)
    Wg = np.asarray(Wg, dtype=np.float32)
    bg = np.asarray(bg, dtype=np.float32)
    n = x.shape[0]

    gates, order = _route(x, Wg, bg)
    key, C, A, per_expert = _schedule(gates, order)
    if key not in _prog_cache:
        _prog_cache[key] = _build_program(key)
    nc = _prog_cache[key]

    tiles, XT, RXT = _plan_tiles(C, A)
    CA = sum(C.values())

    in_maps = []
    core_layout = []   # per core: list of (expert, token_ids, gvals) in CA order
    for c in range(NUM_EXPERTS):
        toks_order = []
        layout = []
        for tier in TIERS:
            ta, ga = per_expert[c][tier]
            tb, gb = per_expert[(c + 1) % NUM_EXPERTS][tier]
            a = A[tier]
            layout.append((c, ta[:a], ga[:a]))
            layout.append(((c + 1) % NUM_EXPERTS, tb[a:], gb[a:]))
            toks_order.append(ta[:a])
            toks_order.append(tb[a:])
        toks = np.concatenate(toks_order)
        assert len(toks) == CA
        xg = np.zeros((CA, D), np.float32)
        real = toks >= 0
        xg[real] = x[toks[real]]
        x8f = _q8(xg)
        rxf = xg - x8f

        def pack_blocks(src, which):
            tot = XT if which == "x" else RXT
            outb = np.zeros((128, tot), f8np)
            for tier, slot, lo, w, xo, rxo, yo in tiles:
                off = xo if which == "x" else rxo
                if which == "rx" and tier < 80:
                    continue
                blk = src[lo:lo + w]                 # [w, 1024]
                b = blk.reshape(w, 4, 2, 128).transpose(3, 1, 2, 0)
                outb[:, off:off + 8 * w] = b.reshape(128, 8 * w).astype(f8np)
            return outb

        w1h_a, w1l_a = _pack_w_dr(W1[c], 32.0)
        w2h_a, w2l_a = _pack_w_dr(W2[c], 64.0)
        cb = (c + 1) % NUM_EXPERTS
        w1h_b, w1l_b = _pack_w_dr(W1[cb], 32.0)
        w2h_b, w2l_b = _pack_w_dr(W2[cb], 64.0)
        bb = np.zeros((128, 2, 16), np.float32)
        for s, e in ((0, c), (1, cb)):
            bb[:, s, 0:8] = 32.0 * b1[e].reshape(8, 128).T
            bb[:, s, 8:16] = 2048.0 * b2[e].reshape(8, 128).T
        in_maps.append({
            "x8": pack_blocks(x8f, "x"),
            "rx8": pack_blocks(rxf, "rx"),
            "w1h": np.stack([w1h_a, w1h_b]),
            "w1l": np.stack([w1l_a, w1l_b]),
            "w2h": np.stack([w2h_a, w2h_b]),
            "w2l": np.stack([w2l_a, w2l_b]),
            "bb": bb,
            "wrm": np.ones((1, 640), np.float32),
        })
        core_layout.append(layout)

    res = bass_utils.run_bass_kernel_spmd(nc, in_maps, list(range(NUM_EXPERTS)))

    out = np.zeros((n, D), np.float32)
    for c in range(NUM_EXPERTS):
        yv = np.asarray(res.results[c]["y"]).astype(np.float32)  # [128, XT]
        # unpack per-tile blocks back to [128, 8, CA] token order
        yfull = np.empty((128, 8, CA), np.float32)
        for tier, slot, lo, w, xo, rxo, yo in tiles:
            yfull[:, :, lo:lo + w] = yv[:, yo:yo + 8 * w].reshape(128, 8, w)
        pos = 0
        for e, toks, gv in core_layout[c]:
            m = toks >= 0
            if m.any():
                idx = np.nonzero(m)[0] + pos
                picked = yfull[:, :, idx]               # [128, 8, len]
                contrib = picked.transpose(2, 1, 0).reshape(-1, D)
                out[toks[m]] += gv[m][:, None].astype(np.float32) * contrib
            pos += len(toks)
        assert pos == CA
    return out
